# revision 1
# baseline (speedup 1.0000x reference)
"""Spiking ConvNet (LIF) kernel for nn_ConvNet_70720931496461.

Data-parallel over batch across the 8 NeuronCores (sharding_hint):
x/state sharded on B (256 -> 8 x 32), small weights replicated, the
sequential scan over T=64 stays local per device. Compiled per-core via
XLA-Neuron through jax.pmap; inputs are taken full, output returned full.
"""
import numpy as np
import jax
import jax.numpy as jnp

# LIF / LI constants (norse defaults, model='super', dt=1e-3)
DT = 1e-3
TAU_SYN_INV = 200.0
TAU_MEM_INV = 100.0
V_TH = 1.0

T, B, NDEV = 64, 256, 8
BL = B // NDEV  # 32 per core


def _conv2d(x, w, b):
    y = jax.lax.conv_general_dilated(
        x, w, window_strides=(1, 1), padding="VALID",
        dimension_numbers=("NCHW", "OIHW", "NCHW"))
    return y + b[None, :, None, None]


def _maxpool2(x):
    return jax.lax.reduce_window(
        x, -jnp.inf, jax.lax.max, (1, 1, 2, 2), (1, 1, 2, 2), "VALID")


def _lif_step(inp, v, i):
    v_dec = v + DT * TAU_MEM_INV * (-v + i)
    i_dec = i * (1.0 - DT * TAU_SYN_INV)
    z = (v_dec - V_TH > 0).astype(v_dec.dtype)
    v_new = (1.0 - z) * v_dec
    i_new = i_dec + inp
    return z, v_new, i_new


def _model(x, w1, b1, w2, b2, w_fc, b_fc, w_out):
    # x: [T, BL, 1, 28, 28] local shard
    dtype = x.dtype
    s0 = (jnp.zeros((BL, 20, 24, 24), dtype), jnp.zeros((BL, 20, 24, 24), dtype))
    s1 = (jnp.zeros((BL, 50, 8, 8), dtype), jnp.zeros((BL, 50, 8, 8), dtype))
    s2 = (jnp.zeros((BL, 500), dtype), jnp.zeros((BL, 500), dtype))
    so = (jnp.zeros((BL, 10), dtype), jnp.zeros((BL, 10), dtype))

    def step(carry, xt):
        (v0, i0), (v1, i1), (v2, i2), (vo, io) = carry
        z = _conv2d(xt, w1, b1)
        z, v0, i0 = _lif_step(z, v0, i0)
        z = _maxpool2(z)
        z = 10.0 * _conv2d(z, w2, b2)
        z, v1, i1 = _lif_step(z, v1, i1)
        z = _maxpool2(z)
        z = z.reshape(BL, -1)
        z = z @ w_fc.T + b_fc
        z, v2, i2 = _lif_step(z, v2, i2)
        v_new = vo + DT * TAU_MEM_INV * (-vo + io)
        i_dec = io * (1.0 - DT * TAU_SYN_INV)
        i_new = i_dec + jax.nn.relu(z) @ w_out.T
        return ((v0, i0), (v1, i1), (v2, i2), (v_new, i_new)), v_new

    _, voltages = jax.lax.scan(step, (s0, s1, s2, so), x)
    return voltages  # [T, BL, 10]


_pmodel = jax.pmap(_model, in_axes=(0,) + (None,) * 7, devices=jax.devices()[:NDEV])


def kernel(x, w1, b1, w2, b2, w_fc, b_fc, w_out):
    x = np.asarray(x, np.float32)
    # [T, B, ...] -> [NDEV, T, BL, ...] batch shard per core
    xs = np.ascontiguousarray(
        x.reshape(T, NDEV, BL, 1, 28, 28).transpose(1, 0, 2, 3, 4, 5))
    out = _pmodel(jnp.asarray(xs),
                  jnp.asarray(w1), jnp.asarray(b1),
                  jnp.asarray(w2), jnp.asarray(b2),
                  jnp.asarray(w_fc), jnp.asarray(b_fc), jnp.asarray(w_out))
    out = np.asarray(jax.device_get(out))          # [NDEV, T, BL, 10]
    out = out.transpose(1, 0, 2, 3).reshape(T, B, 10)
    return np.ascontiguousarray(out.astype(np.float32))

